# revision 65
# baseline (speedup 1.0000x reference)
"""Trainium2 Bass kernel for nn_GAT_1580547974673 (2-layer GAT + pair scoring).

v2: replicated-h design. Every core receives the FULL x (transposed) and
computes h = x @ Wall for ALL N nodes locally in f32r (natural layout), which
eliminates the big [N, 528] AllGather of v1 entirely. Only two small
collectives remain (layer-2 h2 [N,66] and x_out [N,64], ~0.8MB each).

Per core: output rows i in a 384-row slice (8 cores x 384 = 3072), pair
scoring sharded over P (256 pairs/core). Attention is computed transposed
(j on partitions, i on free dim): et[j, i] = exp(lrelu(f1_i + f2_j + m_ji)),
hp^T = [h|1]^T @ et gives hp rows 0..63 and the softmax denominator at PSUM
partition 64 (ones-column LAST in lhsT). The denominator row is recip'd in
place at partition 64 and broadcast to partitions 0..63 with a K=1 matmul
(lhsT = ones column), so no DMA partition-bounces anywhere.

f1/f2 for layer 1 are host-precomputed in fp64 (exact); layer-2 f1/f2 come
from small on-device f32r matmuls. haug/et are bf16 (validated numerically:
~0.5% absmax-rel vs the 2e-2 gate). Total ~14 DMAs vs v1's 164 (the v1
bottleneck was the DMA issue path: ~650ns SP.SEQ + 625ns shared HWDGE each).

elu(t) = relu(t) + min(exp(t), 1) - 1 (single Exp + fused DVE combine).
"""
import numpy as np
from contextlib import ExitStack

import concourse.bass as bass
import concourse.bacc as bacc
import concourse.mybir as mybir
import concourse.tile as tile
import concourse.dve_ops as dve_ops
from concourse.dve_ops import DveOp, OPS
from concourse.dve_spec import Spec, Src0, Src1, C0, C1, One, maxx, minn, relu, lower
from concourse.dve_uop import DveOpSpec
from concourse.bass_utils import run_bass_kernel_spmd
from concourse.masks import make_identity

F32 = mybir.dt.float32
F32R = mybir.dt.float32r
F16 = mybir.dt.float16
BF16 = mybir.dt.bfloat16
AF = mybir.ActivationFunctionType

# problem shapes (hardcoded per spec)
N, FIN, FH, H, NPAIR = 3072, 512, 64, 8, 2048
NC = 8
IB = N // NC            # 384 output rows per core
PB = NPAIR // NC        # 256 pairs per core
NJ = N // 128           # 24 j-blocks
KB = FIN // 128         # 4 k-blocks of the feature dim
SUB = IB // 128         # 3 sub-blocks of the core's row slice
CH = 6                  # j-blocks per exp chunk
NCH = NJ // CH
GB = 4                  # j-blocks per haug tile
MASKVAL = -1.0e5        # alpha*A*(z+M) + B < 0 => Schraudolph clamps to +0.0
ALPHA = 0.2

SIM_NOCOLL = False  # replace collectives with local DMA (for TimelineSim)
STOP_AFTER = None   # 'prep' | 'att1' (truncated builds for phase profiling)

# Schraudolph fast-exp in bf16: exp(y) ~= bitcast_bf16(int16(A*y + B)). All of
# f1, f2 and the mask are pre-scaled by A (host / tiny device matmuls), so one
# DVE op computes the whole et tile: int16(clamp(max(s, alpha*s) + B, 0)),
# s = A*z, written through an int16-bitcast view of the bf16 et tile. Masked
# entries clamp to +0.0 exactly. bf16 (not f32r+int32) because the BIR
# verifier requires f32r matmul inputs to come from rounding producers.
# Validated: 0.63% absmax-rel on scores (gate is 2e-2).
EXP_A = float(2**7 / np.log(2.0))
EXP_B = 16245.0


def _ref_exp_lrelu(in0, in1, s0, s1, imm2):
    s = np.asarray(in0, np.float32) + np.asarray(in1, np.float32) + s0
    w = np.maximum(s, s * s1) + np.float32(imm2)
    return np.maximum(w, np.float32(0.0))


def _register_ops():
    """Register the two custom DVE ops (idempotent)."""
    defs = []
    if "GAT_EXP_LRELU" not in dve_ops._SUB_OPCODE_FOR_NAME:
        s = (Src0 + Src1) + C0
        from concourse.dve_spec import C2, Zero
        defs.append(DveOp(
            "GAT_EXP_LRELU",
            Spec(body=maxx(maxx(s, s * C1) + C2, Zero),
                 reference=_ref_exp_lrelu),
            subdim=False, uops_sha={}))
    if "GAT_ELU_COMBINE" not in dve_ops._SUB_OPCODE_FOR_NAME:
        # out = relu(t) + min(E, 1) - 1  with t=Src0, E=Src1(=exp(t))
        defs.append(DveOp(
            "GAT_ELU_COMBINE",
            Spec(body=relu(Src0) + minn(Src1, One) - One,
                 reference=lambda in0, in1, s0, s1, imm2:
                     np.maximum(in0, 0) + np.minimum(in1, 1.0) - 1.0),
            subdim=False, uops_sha={}))
    for op in defs:
        for ver in ("v3", "v4"):
            tmp = DveOpSpec(name=op.name, opcode=0,
                            uops=lower(op.spec, ver=ver), rd1_en=True)
            op.uops_sha[ver] = tmp.sha(ver)
        dve_ops.OPS.append(op)
        dve_ops.CUSTOM_DVE_SPECS[op.name] = op.spec
        dve_ops._SUB_OPCODE_FOR_NAME[op.name] = (
            dve_ops._CUSTOM_DVE_ROW_BASE + len(dve_ops.OPS) - 1)
    ops = {op.name: op for op in dve_ops.OPS}
    return ops["GAT_EXP_LRELU"], ops["GAT_ELU_COMBINE"]


DBG = {}


def build(nc, reps=1):
    op_exp_lrelu, op_elu = _register_ops()
    I32 = mybir.dt.int32

    # ---- I/O ----
    xT_in = nc.dram_tensor("xT_in", [FIN, N], F32R, kind="ExternalInput")
    Wall_in = nc.dram_tensor("Wall_in", [FIN, H * FH], F32R, kind="ExternalInput")
    Wor_in = nc.dram_tensor("Wor_in", [FH, H * (FH + 2)], F16, kind="ExternalInput")
    wgt_in = nc.dram_tensor("wgt_in", [FH, FH], F32, kind="ExternalInput")
    maskT_in = nc.dram_tensor("maskT_in", [N, IB], mybir.dt.bfloat16,
                              kind="ExternalInput")
    f1b_in = nc.dram_tensor("f1b_in", [128, H * IB], F16, kind="ExternalInput")
    f2n_in = nc.dram_tensor("f2n_in", [N, H], F32, kind="ExternalInput")
    p1T_in = nc.dram_tensor("p1T_in", [N, PB], F16, kind="ExternalInput")
    p2T_in = nc.dram_tensor("p2T_in", [N, PB], F16, kind="ExternalInput")
    scores_out = nc.dram_tensor("scores_out", [1, PB], F32, kind="ExternalOutput")

    groups = [list(range(NC))]

    with tile.TileContext(nc) as tc, ExitStack() as octx:
      for rep in range(reps):
        R = f"_r{rep}"
        ctx = ExitStack()
        octx.enter_context(ctx)
        cst = ctx.enter_context(tc.tile_pool(name="cst" + R, bufs=1))
        ftp = ctx.enter_context(tc.tile_pool(name="ftp" + R, bufs=1))
        maskp = ctx.enter_context(tc.tile_pool(name="maskp" + R, bufs=1))
        hpool = ctx.enter_context(tc.tile_pool(name="hpool" + R, bufs=1))
        ztp = ctx.enter_context(tc.tile_pool(name="ztp" + R, bufs=3))
        npool = ctx.enter_context(tc.tile_pool(name="npool" + R, bufs=2))
        dram = ctx.enter_context(tc.tile_pool(name="dram" + R, bufs=1, space="DRAM"))
        ps_h_pool = ctx.enter_context(tc.tile_pool(name="ps_h" + R, bufs=2, space="PSUM"))
        ps_hp_pool = ctx.enter_context(tc.tile_pool(name="ps_hp" + R, bufs=3, space="PSUM"))
        ps_small = ctx.enter_context(tc.tile_pool(name="ps_small" + R, bufs=2, space="PSUM"))
        ctx_xw = ctx.enter_context(ExitStack())
        xwp = ctx_xw.enter_context(tc.tile_pool(name="xwp" + R, bufs=1))

        # ---- stage A: loads + constants ----
        # DMA order matters: the model serializes transfers, so load what the
        # pipeline front needs first (xT group 0 -> h matmuls start; mask/f1/f2
        # -> DVE starts), then stream the rest behind it.
        XG = 6                 # j-blocks per xT chunk
        xT_g = []
        xT_r = xT_in[:].rearrange("(k p) c -> p k c", p=128)
        for g in range(NJ // XG):
            t = xwp.tile([128, KB, XG * 128], F32R, name=f"xT{g}")
            if g == 0:
                nc.sync.dma_start(t[:], xT_r[:, :, 0:XG * 128])
            xT_g.append(t)
        Wall = xwp.tile([128, KB, H * FH], F32R, name="Wall")
        nc.sync.dma_start(Wall[:], Wall_in[:].rearrange("(k p) c -> p k c", p=128))
        f1ball = ftp.tile([128, H, IB], F16, name="f1ball")
        nc.sync.dma_start(f1ball[:], f1b_in[:].rearrange("p (h c) -> p h c", h=H))
        F2nat = ftp.tile([128, NJ, H], F32, name="F2nat")
        nc.sync.dma_start(F2nat[:], f2n_in[:].rearrange("(g p) c -> p g c", p=128))
        mask_sb = maskp.tile([128, NJ, IB], BF16, name="mask")
        mask_r = maskT_in[:].rearrange("(g p) c -> p g c", p=128)
        nc.sync.dma_start(mask_sb[:, 0:NJ // 2, :], mask_r[:, 0:NJ // 2, :])
        for g in range(1, NJ // XG):
            nc.sync.dma_start(xT_g[g][:], xT_r[:, :, g * XG * 128:(g + 1) * XG * 128])
        nc.sync.dma_start(mask_sb[:, NJ // 2:NJ, :], mask_r[:, NJ // 2:NJ, :])
        Wor = cst.tile([FH, H, FH + 2], F16, name="Wor")
        nc.sync.dma_start(Wor[:], Wor_in[:].rearrange("p (h c) -> p h c", h=H))
        wgt = cst.tile([FH, FH], F32, name="wgt")
        nc.sync.dma_start(wgt[:], wgt_in[:])

        ones16 = cst.tile([128, 128], F16, name="ones16")
        nc.gpsimd.memset(ones16[:], 1.0)
        ones32 = cst.tile([FH + 1, FH], F32, name="ones32")
        nc.gpsimd.memset(ones32[:], 1.0)
        ones_f32 = cst.tile([128, GB * H], F32, name="ones_f32")
        nc.gpsimd.memset(ones_f32[:], 1.0)
        ident = cst.tile([128, 128], F32, name="ident")
        make_identity(nc, ident[:])

        f1b = [f1ball[:, h, :] for h in range(H)]

        # ---- stage C: replicated h for ALL nodes, natural layout ----
        haug_g = []
        for g in range(NJ // GB):
            t = hpool.tile([128, GB, H, FH + 1], BF16, name=f"haug{g}")
            # gpsimd memset of bf16 tiles writes garbage on HW; go through an
            # ACT copy from an f32 ones tile instead (converts on write)
            nc.scalar.copy(t[:, :, :, FH],
                           ones_f32[:].rearrange("p (a b) -> p a b", a=GB))
            haug_g.append(t)

        def haug_lhsT(jb, h):
            return haug_g[jb // GB][:, jb % GB, h, :]

        for jb in range(NJ):
            ps_h = ps_h_pool.tile([128, H * FH], F32, tag="ph", name=f"psh{jb}")
            xs = xT_g[jb // XG]
            for kb in range(KB):
                nc.tensor.matmul(ps_h[:], xs[:, kb, (jb % XG) * 128:(jb % XG + 1) * 128],
                                 Wall[:, kb, :],
                                 start=(kb == 0), stop=(kb == KB - 1))
            nc.scalar.copy(
                haug_g[jb // GB][:, jb % GB, :, 0:FH],
                ps_h[:].rearrange("p (h f) -> p h f", h=H))

        ctx_xw.close()
        elup = ctx.enter_context(tc.tile_pool(name="elup" + R, bufs=1))
        h2p = ctx.enter_context(tc.tile_pool(name="h2p" + R, bufs=1))
        ppool = ctx.enter_context(tc.tile_pool(name="ppool" + R, bufs=1))
        epool = ctx.enter_context(tc.tile_pool(name="epool" + R, bufs=1))

        # issued from the ACT queue: their region-reuse WAR (they reuse the xT
        # region) must not block SP's later gather/output DMAs
        p1r = ppool.tile([128, NJ, PB], F16, name="p1r")
        nc.scalar.dma_start(p1r[:], p1T_in[:].rearrange("(g p) c -> p g c", p=128))
        p2r = ppool.tile([128, NJ, PB], F16, name="p2r")
        nc.scalar.dma_start(p2r[:], p2T_in[:].rearrange("(g p) c -> p g c", p=128))

        if STOP_AFTER == "prep":
            srow0 = npool.tile([1, PB], F32, tag="sr0")
            nc.vector.tensor_copy(srow0[:], mask_sb[0:1, 0, 0:PB])
            nc.sync.dma_start(scores_out[:], srow0[:])
            ctx.close()
            continue

        # ---- attention unit (shared by the 8 heads and the layer-2 pass) ----
        def attention(tag, f1b_t, lhsT_of, s0_of, out_tile):
            """out_tile[0:FH, :] = elu((att @ [h|1])[0:FH] / rowsum)."""
            ps_hp = ps_hp_pool.tile([FH + 1, IB], F32, tag="hp", name=f"hp{tag}")
            for c in range(NCH):
                et = ztp.tile([128, CH, IB], BF16, tag="zt", name=f"et{tag}_{c}")
                for g in range(CH):
                    jb = c * CH + g
                    nc.vector._custom_dve(
                        op_exp_lrelu, out=et[:, g, :].bitcast(mybir.dt.int16),
                        in0=f1b_t[:], in1=mask_sb[:, jb, :], s0=s0_of(jb),
                        s1=ALPHA, imm2=EXP_B)
                for g in range(CH):
                    jb = c * CH + g
                    nc.tensor.matmul(ps_hp[:], lhsT_of(jb), et[:, g, :],
                                     start=(jb == 0), stop=(jb == NJ - 1))
            # denominator lives at PSUM partition FH; recip in place at that
            # partition, then matmul-broadcast (K=1) down to partitions 0..63
            sr = npool.tile([FH + 1, IB], F32, tag="sr", name=f"sr{tag}")
            nc.scalar.copy(sr[:], ps_hp[:])
            rr = npool.tile([FH + 1, IB], F32, tag="rr", name=f"rr{tag}")
            # custom-DVE ops misbehave at partition base 64 on HW; run the
            # recip over the whole 65-row tile (base 0) and use row 64 only
            nc.vector.reciprocal_approx_fast(rr[:], sr[:])
            # fp32 K=1 matmul: 1/rs spans ~1e-14..1e-3, far outside fp16 range
            ps_rb = ps_small.tile([FH, IB], F32, tag="pss", name=f"psrb{tag}")
            nc.tensor.matmul(ps_rb[:], ones32[FH:FH + 1, :],
                             rr[FH:FH + 1, :],
                             start=True, stop=True)
            rb = npool.tile([FH, IB], F32, tag="rb", name=f"rb{tag}")
            nc.scalar.copy(rb[:], ps_rb[:])
            t_n = npool.tile([FH, IB], F32, tag="tn", name=f"tn{tag}")
            nc.vector.tensor_mul(t_n[:], ps_hp[0:FH, :], rb[:])
            e_n = npool.tile([FH, IB], F32, tag="en", name=f"en{tag}")
            nc.scalar.activation(e_n[:], t_n[:], AF.Exp)
            nc.vector._custom_dve(op_elu, out=out_tile[:], in0=t_n[:], in1=e_n[:])

        # ---- stage D: the 8 heads ----
        eluo = []
        for h in range(H):
            t = elup.tile([FH, IB], F16, name=f"eluo{h}")
            eluo.append(t)
        for h in range(H):
            attention(f"h{h}", f1b[h],
                      lambda jb, h=h: haug_lhsT(jb, h),
                      lambda jb, h=h: F2nat[:, jb, h:h + 1],
                      eluo[h])

        if STOP_AFTER == "att1":
            srow0 = npool.tile([1, PB], F32, tag="sr0")
            nc.vector.tensor_copy(srow0[:], eluo[0][0:1, 0:PB])
            nc.sync.dma_start(scores_out[:], srow0[:])
            ctx.close()
            continue

        # ---- stage E: layer-2 h2 (local rows) + f12, then gather ----
        # h2loc cols: [h2(0:64) | ones(64) | A*f2_2(65)]
        h2loc = h2p.tile([128, SUB, FH + 2], F32, name="h2loc")
        nc.gpsimd.memset(h2loc[:, :, FH], 1.0)
        for s in range(SUB):
            ps_h2 = ps_small.tile([128, FH + 1], F32, tag="pss", name=f"psh2{s}")
            for h in range(H):
                nc.tensor.matmul(ps_h2[:], eluo[h][:, s * 128:(s + 1) * 128],
                                 Wor[:, h, 0:FH + 1],
                                 start=(h == 0), stop=(h == H - 1))
            nc.scalar.copy(h2loc[:, s, 0:FH], ps_h2[:, 0:FH])
            # f2_2 column pre-scaled by A for the Schraudolph op
            nc.scalar.mul(h2loc[:, s, FH + 1:FH + 2], ps_h2[:, FH:FH + 1], EXP_A)
        ps_f12 = ps_small.tile([1, IB], F32, tag="pss", name="psf12")
        for h in range(H):
            nc.tensor.matmul(ps_f12[:], Wor[:, h, FH + 1:FH + 2], eluo[h][:],
                             start=(h == 0), stop=(h == H - 1))
        # A-prescale f12 on the copy out of PSUM (fp16 for the K=1 broadcast)
        f12sb = h2p.tile([1, IB], F16, name="f12sb")
        nc.scalar.mul(f12sb[:], ps_f12[:], EXP_A)
        ps_fb = ps_small.tile([128, IB], F32, tag="pss", name="psfb")
        nc.tensor.matmul(ps_fb[:], ones16[0:1, 0:128], f12sb[:],
                         start=True, stop=True)
        f12b = h2p.tile([128, IB], F16, name="f12b")
        nc.scalar.copy(f12b[:], ps_fb[:])

        h2loc_d = dram.tile([IB, FH + 2], F32)
        nc.sync.dma_start(h2loc_d[:].rearrange("(s p) c -> p s c", p=128), h2loc[:])
        h2g_d = dram.tile([N, FH + 2], F32, addr_space="Shared")
        if SIM_NOCOLL:
            nc.sync.dma_start(h2g_d[0:IB, :], h2loc_d[:])
        else:
            nc.gpsimd.collective_compute(
                "AllGather", mybir.AluOpType.bypass, replica_groups=groups,
                ins=[h2loc_d[:].opt()], outs=[h2g_d[:].opt()])
        h2r = h2p.tile([128, NJ, FH + 2], F32, name="h2r")
        nc.sync.dma_start(h2r[:], h2g_d[:].rearrange("(g p) c -> p g c", p=128))
        # bf16 view of [h2|1] for the layer-2 attention matmuls
        h2b = h2p.tile([128, NJ, FH + 1], BF16, name="h2b")
        nc.scalar.copy(h2b[:], h2r[:, :, 0:FH + 1])

        # ---- stage F: layer-2 attention (single head) ----
        xoT = h2p.tile([FH, IB], F32, name="xoT")
        attention("l2", f12b,
                  lambda jb: h2b[:, jb, :],
                  lambda jb: h2r[:, jb, FH + 1:FH + 2],
                  xoT)

        # ---- stage G: x_out natural layout + gather ----
        xol = h2p.tile([128, SUB, FH], F16, name="xol")
        for s in range(SUB):
            ps_tr = ps_small.tile([128, FH], F32, tag="pss", name=f"pstr{s}")
            nc.tensor.transpose(ps_tr[:], xoT[:, s * 128:(s + 1) * 128],
                                ident[0:FH, 0:FH])
            nc.scalar.copy(xol[:, s, :], ps_tr[:])
        xo_d = dram.tile([IB, FH], F16)
        nc.sync.dma_start(xo_d[:].rearrange("(s p) c -> p s c", p=128), xol[:])
        xog_d = dram.tile([N, FH], F16, addr_space="Shared")
        if SIM_NOCOLL:
            nc.sync.dma_start(xog_d[0:IB, :], xo_d[:])
        else:
            nc.gpsimd.collective_compute(
                "AllGather", mybir.AluOpType.bypass, replica_groups=groups,
                ins=[xo_d[:].opt()], outs=[xog_d[:].opt()])
        xor_ = h2p.tile([128, NJ, FH], F16, name="xor")
        nc.sync.dma_start(xor_[:], xog_d[:].rearrange("(g p) c -> p g c", p=128))
        DBG["h2g"] = h2g_d
        DBG["xog"] = xog_d
        DBG["eluo"] = eluo
        DBG["haug0"] = haug_g[0]
        DBG["f1ball"] = f1ball
        DBG["F2nat"] = F2nat
        DBG["mask"] = mask_sb

        # ---- stage H: pair embeddings + scores ----
        ps_e1 = ps_small.tile([FH, PB], F32, tag="pss", name="ps_e1")
        for jb in range(NJ):
            nc.tensor.matmul(ps_e1[:], xor_[:, jb, :], p1r[:, jb, :],
                             start=(jb == 0), stop=(jb == NJ - 1))
        e1sb = epool.tile([FH, PB], F32)
        nc.scalar.copy(e1sb[:], ps_e1[:])
        ps_e2 = ps_small.tile([FH, PB], F32, tag="pss", name="ps_e2")
        for jb in range(NJ):
            nc.tensor.matmul(ps_e2[:], xor_[:, jb, :], p2r[:, jb, :],
                             start=(jb == 0), stop=(jb == NJ - 1))
        e2sb = epool.tile([FH, PB], F32)
        nc.scalar.copy(e2sb[:], ps_e2[:])

        ps_g = ps_small.tile([FH, PB], F32, tag="pss", name="ps_g")
        nc.tensor.matmul(ps_g[:], wgt[:], e1sb[:], start=True, stop=True)
        prod = epool.tile([FH, PB], F32)
        nc.vector.tensor_mul(prod[:], ps_g[:], e2sb[:])
        ps_s = ps_small.tile([1, PB], F32, tag="pss", name="ps_s")
        nc.tensor.matmul(ps_s[:], ones32[0:FH, 0:1], prod[:], start=True, stop=True)
        srow = epool.tile([1, PB], F32)
        nc.scalar.copy(srow[:], ps_s[:])
        nc.sync.dma_start(scores_out[:], srow[:])
        ctx.close()

    return nc


_CACHE = {}


def _get_nc(reps=1):
    key = f"nc{reps}"
    if key not in _CACHE:
        nc = bacc.Bacc(None, target_bir_lowering=False, debug=False, num_devices=NC)
        build(nc, reps=reps)
        nc.compile()
        _CACHE[key] = nc
    return _CACHE[key]


def prep_inputs(x, adj, pair1_map, pair2_map, Wh, a1h, a2h, W_out, a1_out,
                a2_out, weight):
    import concourse.mybir as _mb
    bf16 = _mb.dt.np(_mb.dt.bfloat16)
    x64 = np.asarray(x, np.float64)
    adj = np.asarray(adj)
    xT = np.ascontiguousarray(np.asarray(x, np.float32).T)             # [FIN, N]
    # mask pre-scaled by the Schraudolph A; any hugely-negative value works
    maskT = np.where(adj > 0, np.float32(0.0),
                     np.float32(EXP_A * MASKVAL)).T.astype(bf16)       # [j, i]
    Wall = np.ascontiguousarray(
        np.transpose(np.asarray(Wh, np.float64), (1, 0, 2)).reshape(FIN, H * FH)
    ).astype(np.float32)
    # layer-1 f1/f2 host-exact in fp64
    w1 = np.einsum("hkf,hf->kh", np.asarray(Wh, np.float64), np.asarray(a1h, np.float64))
    w2 = np.einsum("hkf,hf->kh", np.asarray(Wh, np.float64), np.asarray(a2h, np.float64))
    F1 = x64 @ w1                                                      # [N, H]
    F2 = x64 @ w2
    FT1 = np.ascontiguousarray((F1 * EXP_A).T.astype(np.float16))      # A*f1, [H, N]
    f2n = np.ascontiguousarray((F2 * EXP_A).astype(np.float32))        # A*f2, [N, H]
    # layer-2 weights, head-reordered: Wor[f, h, :] = [W_out[h*64+f] | w2o | w1o]
    w1o = np.asarray(W_out, np.float64) @ np.asarray(a1_out, np.float64)
    w2o = np.asarray(W_out, np.float64) @ np.asarray(a2_out, np.float64)
    Wof = np.concatenate([np.asarray(W_out, np.float64), w2o[:, None],
                          w1o[:, None]], axis=1)                       # [512, 66]
    Wor = np.ascontiguousarray(
        Wof.reshape(H, FH, FH + 2).transpose(1, 0, 2).reshape(FH, H * (FH + 2))
    ).astype(np.float16)
    p1T = np.ascontiguousarray(np.asarray(pair1_map, np.float16).T)    # [N, NPAIR]
    p2T = np.ascontiguousarray(np.asarray(pair2_map, np.float16).T)
    wgt = np.ascontiguousarray(np.asarray(weight, np.float32))

    in_maps = []
    for c in range(NC):
        i0, i1 = c * IB, (c + 1) * IB
        p0, p1 = c * PB, (c + 1) * PB
        in_maps.append({
            "xT_in": xT,
            "Wall_in": Wall,
            "Wor_in": Wor,
            "wgt_in": wgt,
            "maskT_in": np.ascontiguousarray(maskT[:, i0:i1]),
            "f1b_in": np.ascontiguousarray(np.broadcast_to(
                FT1[:, i0:i1].reshape(1, H * IB), (128, H * IB))),
            "f2n_in": f2n,
            "p1T_in": np.ascontiguousarray(p1T[:, p0:p1]),
            "p2T_in": np.ascontiguousarray(p2T[:, p0:p1]),
        })
    return in_maps


def run(inputs, trace=False, **kw):
    nc = _get_nc()
    in_maps = prep_inputs(**inputs)
    res = run_bass_kernel_spmd(nc, in_maps, list(range(NC)), trace=trace, **kw)
    scores = np.concatenate(
        [res.results[c]["scores_out"].reshape(-1) for c in range(NC)])
    return scores.astype(np.float32), res


def kernel(**inputs):
    return run(inputs)[0]


def _make_fn(nc, in_maps):
    import jax
    from jax.sharding import Mesh, PartitionSpec, NamedSharding
    from jax.experimental.shard_map import shard_map
    from concourse import bass2jax
    import concourse.mybir as _mb

    bass2jax.install_neuronx_cc_hook()
    partition_name = nc.partition_id_tensor.name if nc.partition_id_tensor else None
    in_names, out_names, out_avals, zero_outs = [], [], [], []
    for alloc in nc.m.functions[0].allocations:
        if not isinstance(alloc, _mb.MemoryLocationSet):
            continue
        name = alloc.memorylocations[0].name
        if alloc.kind == "ExternalInput":
            if name != partition_name:
                in_names.append(name)
        elif alloc.kind == "ExternalOutput":
            shape = list(alloc.tensor_shape)
            npdt = _mb.dt.np(alloc.dtype)
            out_names.append(name)
            out_avals.append(jax.core.ShapedArray(shape, npdt))
            zero_outs.append(np.zeros(shape, npdt))
    n_params = len(in_names)
    n_outs = len(out_names)
    all_in_names = list(in_names) + list(out_names)
    if partition_name is not None:
        all_in_names.append(partition_name)

    def _body(*args):
        operands = list(args)
        if partition_name is not None:
            operands.append(bass2jax.partition_id_tensor())
        outs = bass2jax._bass_exec_p.bind(
            *operands, out_avals=tuple(out_avals), in_names=tuple(all_in_names),
            out_names=tuple(out_names), lowering_input_output_aliases=(),
            sim_require_finite=True, sim_require_nnan=True, nc=nc)
        return tuple(outs)

    devices = jax.devices()[:NC]
    mesh = Mesh(np.asarray(devices), ("core",))
    in_specs = (PartitionSpec("core"),) * (n_params + n_outs)
    out_specs = (PartitionSpec("core"),) * n_outs
    fn = jax.jit(shard_map(_body, mesh=mesh, in_specs=in_specs,
                           out_specs=out_specs, check_rep=False),
                 keep_unused=True)
    concat_in = [
        np.concatenate([np.asarray(in_maps[c][nm]) for c in range(NC)], axis=0)
        for nm in in_names]
    concat_zeros = [np.zeros((NC * z.shape[0], *z.shape[1:]), z.dtype)
                    for z in zero_outs]
    sh = NamedSharding(mesh, PartitionSpec("core"))
    dev_in = [jax.device_put(a, sh) for a in concat_in]
    dev_zero = [jax.device_put(a, sh) for a in concat_zeros]
    return fn, dev_in, dev_zero


def bench(inputs, iters=6, kreps=5):
    """Device time per kernel pass, via the in-NEFF replication slope.

    Builds the program once with 1 rep and once with `kreps` reps of the
    whole computation; (t_k - t_1)/(k - 1) cancels the (large, stable) axon
    dispatch floor and yields per-pass device time.
    """
    import time
    import jax
    in_maps = prep_inputs(**inputs)
    fns = {}
    for reps in (1, kreps):
        nc = _get_nc(reps=reps)
        fn, dev_in, dev_zero = _make_fn(nc, in_maps)
        jax.block_until_ready(fn(*dev_in, *dev_zero))  # warm/compile
        fns[reps] = (fn, dev_in, dev_zero)

    def once(reps):
        fn, dev_in, dev_zero = fns[reps]
        t0 = time.perf_counter()
        jax.block_until_ready(fn(*dev_in, *dev_zero))
        return time.perf_counter() - t0

    t1s, tks, diffs = [], [], []
    for _ in range(3 * iters):
        a = once(1)
        b = once(kreps)
        c = once(1)
        t1s += [a, c]
        tks.append(b)
        diffs.append(b - (a + c) / 2)
    diffs.sort()
    med = diffs[len(diffs) // 2]
    out = {
        "t1_ns": min(t1s) * 1e9,
        f"t{kreps}_ns": min(tks) * 1e9,
        "pooled_med_ns": med / (kreps - 1) * 1e9,
        "per_exec_ns": max(med / (kreps - 1) * 1e9, 0.0),
    }
    return out


if __name__ == "__main__":
    # quick self-drive with random inputs of the right shapes (no reference)
    rng = np.random.default_rng(0)
    ins = dict(
        x=rng.standard_normal((N, FIN), dtype=np.float32),
        adj=(rng.random((N, N)) < 0.5).astype(np.int32),
        pair1_map=rng.standard_normal((NPAIR, N), dtype=np.float32),
        pair2_map=rng.standard_normal((NPAIR, N), dtype=np.float32),
        Wh=rng.standard_normal((H, FIN, FH), dtype=np.float32) * 0.1,
        a1h=rng.standard_normal((H, FH), dtype=np.float32) * 0.3,
        a2h=rng.standard_normal((H, FH), dtype=np.float32) * 0.3,
        W_out=rng.standard_normal((FIN, FH), dtype=np.float32) * 0.1,
        a1_out=rng.standard_normal((FH,), dtype=np.float32) * 0.3,
        a2_out=rng.standard_normal((FH,), dtype=np.float32) * 0.3,
        weight=rng.standard_normal((FH, FH), dtype=np.float32) * 0.1,
    )
    out = kernel(**ins)
    print("scores:", out.shape, out[:8])


# revision 74
# speedup vs baseline: 1.5860x; 1.5860x over previous
"""Trainium2 Bass kernel for nn_GAT_1580547974673 (2-layer GAT + pair scoring).

v2: replicated-h design. Every core receives the FULL x (transposed) and
computes h = x @ Wall for ALL N nodes locally in f32r (natural layout), which
eliminates the big [N, 528] AllGather of v1 entirely. Only two small
collectives remain (layer-2 h2 [N,66] and x_out [N,64], ~0.8MB each).

Per core: output rows i in a 384-row slice (8 cores x 384 = 3072), pair
scoring sharded over P (256 pairs/core). Attention is computed transposed
(j on partitions, i on free dim): et[j, i] = exp(lrelu(f1_i + f2_j + m_ji)),
hp^T = [h|1]^T @ et gives hp rows 0..63 and the softmax denominator at PSUM
partition 64 (ones-column LAST in lhsT). The denominator row is recip'd in
place at partition 64 and broadcast to partitions 0..63 with a K=1 matmul
(lhsT = ones column), so no DMA partition-bounces anywhere.

f1/f2 for layer 1 are host-precomputed in fp64 (exact); layer-2 f1/f2 come
from small on-device f32r matmuls. haug/et are bf16 (validated numerically:
~0.5% absmax-rel vs the 2e-2 gate). Total ~14 DMAs vs v1's 164 (the v1
bottleneck was the DMA issue path: ~650ns SP.SEQ + 625ns shared HWDGE each).

elu(t) = relu(t) + min(exp(t), 1) - 1 (single Exp + fused DVE combine).
"""
import numpy as np
from contextlib import ExitStack

import concourse.bass as bass
import concourse.bacc as bacc
import concourse.mybir as mybir
import concourse.tile as tile
import concourse.dve_ops as dve_ops
from concourse.dve_ops import DveOp, OPS
from concourse.dve_spec import Spec, Src0, Src1, C0, C1, One, maxx, minn, relu, lower
from concourse.dve_uop import DveOpSpec
from concourse.bass_utils import run_bass_kernel_spmd
from concourse.masks import make_identity

F32 = mybir.dt.float32
F32R = mybir.dt.float32r
F16 = mybir.dt.float16
BF16 = mybir.dt.bfloat16
AF = mybir.ActivationFunctionType

# problem shapes (hardcoded per spec)
N, FIN, FH, H, NPAIR = 3072, 512, 64, 8, 2048
NC = 8
IB = N // NC            # 384 output rows per core
PB = NPAIR // NC        # 256 pairs per core
NJ = N // 128           # 24 j-blocks
KB = FIN // 128         # 4 k-blocks of the feature dim
SUB = IB // 128         # 3 sub-blocks of the core's row slice
CH = 6                  # j-blocks per exp chunk
NCH = NJ // CH
GB = 4                  # j-blocks per haug tile
MASKVAL = -1.0e5        # alpha*A*(z+M) + B < 0 => Schraudolph clamps to +0.0
ALPHA = 0.2

SIM_NOCOLL = False  # replace collectives with local DMA (for TimelineSim)
STOP_AFTER = None   # 'prep' | 'att1' (truncated builds for phase profiling)

# Schraudolph fast-exp in bf16: exp(y) ~= bitcast_bf16(int16(A*y + B)). All of
# f1, f2 and the mask are pre-scaled by A (host / tiny device matmuls), so one
# DVE op computes the whole et tile: int16(clamp(max(s, alpha*s) + B, 0)),
# s = A*z, written through an int16-bitcast view of the bf16 et tile. Masked
# entries clamp to +0.0 exactly. bf16 (not f32r+int32) because the BIR
# verifier requires f32r matmul inputs to come from rounding producers.
# Validated: 0.63% absmax-rel on scores (gate is 2e-2).
EXP_A = float(2**7 / np.log(2.0))
EXP_B = 16245.0


def _ref_exp_lrelu(in0, in1, s0, s1, imm2):
    s = np.asarray(in0, np.float32) + np.asarray(in1, np.float32) + s0
    w = np.maximum(s, s * s1) + np.float32(imm2)
    return np.maximum(w, np.float32(0.0))


def _register_ops():
    """Register the two custom DVE ops (idempotent)."""
    defs = []
    if "GAT_EXP_LRELU" not in dve_ops._SUB_OPCODE_FOR_NAME:
        s = (Src0 + Src1) + C0
        from concourse.dve_spec import C2, Zero
        defs.append(DveOp(
            "GAT_EXP_LRELU",
            Spec(body=maxx(maxx(s, s * C1) + C2, Zero),
                 reference=_ref_exp_lrelu),
            subdim=False, uops_sha={}))
    if "GAT_ELU_COMBINE" not in dve_ops._SUB_OPCODE_FOR_NAME:
        # out = relu(t) + min(E, 1) - 1  with t=Src0, E=Src1(=exp(t))
        defs.append(DveOp(
            "GAT_ELU_COMBINE",
            Spec(body=relu(Src0) + minn(Src1, One) - One,
                 reference=lambda in0, in1, s0, s1, imm2:
                     np.maximum(in0, 0) + np.minimum(in1, 1.0) - 1.0),
            subdim=False, uops_sha={}))
    for op in defs:
        for ver in ("v3", "v4"):
            tmp = DveOpSpec(name=op.name, opcode=0,
                            uops=lower(op.spec, ver=ver), rd1_en=True)
            op.uops_sha[ver] = tmp.sha(ver)
        dve_ops.OPS.append(op)
        dve_ops.CUSTOM_DVE_SPECS[op.name] = op.spec
        dve_ops._SUB_OPCODE_FOR_NAME[op.name] = (
            dve_ops._CUSTOM_DVE_ROW_BASE + len(dve_ops.OPS) - 1)
    ops = {op.name: op for op in dve_ops.OPS}
    return ops["GAT_EXP_LRELU"], ops["GAT_ELU_COMBINE"]


DBG = {}


def build(nc, reps=1):
    op_exp_lrelu, op_elu = _register_ops()
    I32 = mybir.dt.int32

    # ---- I/O ----
    xT_in = nc.dram_tensor("xT_in", [FIN, N], F32R, kind="ExternalInput")
    Wall_in = nc.dram_tensor("Wall_in", [FIN, H * FH], F32R, kind="ExternalInput")
    Wor_in = nc.dram_tensor("Wor_in", [FH, H * (FH + 2)], F16, kind="ExternalInput")
    wgt_in = nc.dram_tensor("wgt_in", [FH, FH], F32, kind="ExternalInput")
    maskT_in = nc.dram_tensor("maskT_in", [N, IB], mybir.dt.bfloat16,
                              kind="ExternalInput")
    f1b_in = nc.dram_tensor("f1b_in", [128, H * IB], F16, kind="ExternalInput")
    f2n_in = nc.dram_tensor("f2n_in", [N, H], F32, kind="ExternalInput")
    p1T_in = nc.dram_tensor("p1T_in", [N, PB], F16, kind="ExternalInput")
    p2T_in = nc.dram_tensor("p2T_in", [N, PB], F16, kind="ExternalInput")
    scores_out = nc.dram_tensor("scores_out", [1, PB], F32, kind="ExternalOutput")

    groups = [list(range(NC))]

    with tile.TileContext(nc) as tc, ExitStack() as octx:
      for rep in range(reps):
        R = f"_r{rep}"
        ctx = ExitStack()
        octx.enter_context(ctx)
        cst = ctx.enter_context(tc.tile_pool(name="cst" + R, bufs=1))
        ftp = ctx.enter_context(tc.tile_pool(name="ftp" + R, bufs=1))
        maskp = ctx.enter_context(tc.tile_pool(name="maskp" + R, bufs=1))
        hpool = ctx.enter_context(tc.tile_pool(name="hpool" + R, bufs=1))
        ztp = ctx.enter_context(tc.tile_pool(name="ztp" + R, bufs=3))
        npool = ctx.enter_context(tc.tile_pool(name="npool" + R, bufs=2))
        dram = ctx.enter_context(tc.tile_pool(name="dram" + R, bufs=1, space="DRAM"))
        ps_h_pool = ctx.enter_context(tc.tile_pool(name="ps_h" + R, bufs=2, space="PSUM"))
        ps_hp_pool = ctx.enter_context(tc.tile_pool(name="ps_hp" + R, bufs=3, space="PSUM"))
        ps_small = ctx.enter_context(tc.tile_pool(name="ps_small" + R, bufs=2, space="PSUM"))
        ctx_xw = ctx.enter_context(ExitStack())
        xwp = ctx_xw.enter_context(tc.tile_pool(name="xwp" + R, bufs=1))

        # ---- stage A: loads + constants ----
        # DMA order matters: the model serializes transfers, so load what the
        # pipeline front needs first (xT group 0 -> h matmuls start; mask/f1/f2
        # -> DVE starts), then stream the rest behind it.
        XG = 6                 # j-blocks per xT chunk
        xT_g = []
        xT_r = xT_in[:].rearrange("(k p) c -> p k c", p=128)
        for g in range(NJ // XG):
            t = xwp.tile([128, KB, XG * 128], F32R, name=f"xT{g}")
            if g == 0:
                nc.sync.dma_start(t[:], xT_r[:, :, 0:XG * 128])
            xT_g.append(t)
        Wall = xwp.tile([128, KB, H * FH], F32R, name="Wall")
        nc.sync.dma_start(Wall[:], Wall_in[:].rearrange("(k p) c -> p k c", p=128))
        f1ball = ftp.tile([128, H, IB], F16, name="f1ball")
        nc.sync.dma_start(f1ball[:], f1b_in[:].rearrange("p (h c) -> p h c", h=H))
        F2nat = ftp.tile([128, NJ, H], F32, name="F2nat")
        nc.sync.dma_start(F2nat[:], f2n_in[:].rearrange("(g p) c -> p g c", p=128))
        mask_sb = maskp.tile([128, NJ, IB], BF16, name="mask")
        nc.sync.dma_start(mask_sb[:], maskT_in[:].rearrange("(g p) c -> p g c", p=128))
        for g in range(1, NJ // XG):
            nc.sync.dma_start(xT_g[g][:], xT_r[:, :, g * XG * 128:(g + 1) * XG * 128])
        Wor = cst.tile([FH, H, FH + 2], F16, name="Wor")
        nc.sync.dma_start(Wor[:], Wor_in[:].rearrange("p (h c) -> p h c", h=H))
        wgt = cst.tile([FH, FH], F32, name="wgt")
        nc.sync.dma_start(wgt[:], wgt_in[:])

        ones16 = cst.tile([128, 128], F16, name="ones16")
        nc.gpsimd.memset(ones16[:], 1.0)
        ones32 = cst.tile([FH + 1, FH], F32, name="ones32")
        nc.gpsimd.memset(ones32[:], 1.0)
        ones_f32 = cst.tile([128, GB * H], F32, name="ones_f32")
        nc.gpsimd.memset(ones_f32[:], 1.0)
        ident = cst.tile([128, 128], F32, name="ident")
        make_identity(nc, ident[:])

        f1b = [f1ball[:, h, :] for h in range(H)]

        # ---- stage C: replicated h for ALL nodes, natural layout ----
        haug_g = []
        for g in range(NJ // GB):
            t = hpool.tile([128, GB, H, FH + 1], BF16, name=f"haug{g}")
            # gpsimd memset of bf16 tiles writes garbage on HW; go through an
            # ACT copy from an f32 ones tile instead (converts on write)
            nc.scalar.copy(t[:, :, :, FH],
                           ones_f32[:].rearrange("p (a b) -> p a b", a=GB))
            haug_g.append(t)

        def haug_lhsT(jb, h):
            return haug_g[jb // GB][:, jb % GB, h, :]

        # ---- attention unit (shared by the 8 heads and the layer-2 pass) ----
        def att_chunk(tag, c, ps_hp, f1b_t, lhsT_of, s0_of):
            et = ztp.tile([128, CH, IB], BF16, tag="zt", name=f"et{tag}_{c}")
            for g in range(CH):
                jb = c * CH + g
                nc.vector._custom_dve(
                    op_exp_lrelu, out=et[:, g, :].bitcast(mybir.dt.int16),
                    in0=f1b_t[:], in1=mask_sb[:, jb, :], s0=s0_of(jb),
                    s1=ALPHA, imm2=EXP_B)
            for g in range(CH):
                jb = c * CH + g
                nc.tensor.matmul(ps_hp[:], lhsT_of(jb), et[:, g, :],
                                 start=(jb == 0), stop=(jb == NJ - 1))

        def attention(tag, f1b_t, lhsT_of, s0_of, out_tile, skip_chunks=0,
                      ps_hp=None):
            """out_tile[0:FH, :] = elu((att @ [h|1])[0:FH] / rowsum)."""
            if ps_hp is None:
                ps_hp = ps_hp_pool.tile([FH + 1, IB], F32, tag="hp",
                                        name=f"hp{tag}")
            for c in range(skip_chunks, NCH):
                att_chunk(tag, c, ps_hp, f1b_t, lhsT_of, s0_of)
            # denominator lives at PSUM partition FH; recip in place at that
            # partition, then matmul-broadcast (K=1) down to partitions 0..63
            sr = npool.tile([FH + 1, IB], F32, tag="sr", name=f"sr{tag}")
            nc.scalar.copy(sr[:], ps_hp[:])
            rr = npool.tile([FH + 1, IB], F32, tag="rr", name=f"rr{tag}")
            # custom-DVE ops misbehave at partition base 64 on HW; run the
            # recip over the whole 65-row tile (base 0) and use row 64 only
            nc.vector.reciprocal_approx_fast(rr[:], sr[:])
            # fp32 K=1 matmul: 1/rs spans ~1e-14..1e-3, far outside fp16 range
            ps_rb = ps_small.tile([FH, IB], F32, tag="pss", name=f"psrb{tag}")
            nc.tensor.matmul(ps_rb[:], ones32[FH:FH + 1, :],
                             rr[FH:FH + 1, :],
                             start=True, stop=True)
            rb = npool.tile([FH, IB], F32, tag="rb", name=f"rb{tag}")
            nc.scalar.copy(rb[:], ps_rb[:])
            # multiply on gpsimd (SBUF-only) to keep the bottleneck DVE free;
            # hp rows come via the same sr copy (rows 0:64)
            t_n = npool.tile([FH, IB], F32, tag="tn", name=f"tn{tag}")
            nc.gpsimd.tensor_mul(t_n[:], sr[0:FH, :], rb[:])
            e_n = npool.tile([FH, IB], F32, tag="en", name=f"en{tag}")
            nc.scalar.activation(e_n[:], t_n[:], AF.Exp)
            nc.vector._custom_dve(op_elu, out=out_tile[:], in0=t_n[:], in1=e_n[:])

        # Interleave head 0's attention chunks with h production so the PE's
        # static schedule alternates producer/consumer instead of running all
        # 96 h matmuls first (which stalls the DVE once ztp fills). CH == XG.
        hp0 = ps_hp_pool.tile([FH + 1, IB], F32, tag="hp", name="hp_h0")
        for c in range(NCH):
            for jb in range(c * CH, (c + 1) * CH):
                ps_h = ps_h_pool.tile([128, H * FH], F32, tag="ph", name=f"psh{jb}")
                xs = xT_g[jb // XG]
                for kb in range(KB):
                    nc.tensor.matmul(
                        ps_h[:], xs[:, kb, (jb % XG) * 128:(jb % XG + 1) * 128],
                        Wall[:, kb, :],
                        start=(kb == 0), stop=(kb == KB - 1))
                nc.scalar.copy(
                    haug_g[jb // GB][:, jb % GB, :, 0:FH],
                    ps_h[:].rearrange("p (h f) -> p h f", h=H))
            att_chunk("h0", c, hp0, f1b[0],
                      lambda jb: haug_lhsT(jb, 0),
                      lambda jb: F2nat[:, jb, 0:1])

        ctx_xw.close()
        elup = ctx.enter_context(tc.tile_pool(name="elup" + R, bufs=1))
        h2p = ctx.enter_context(tc.tile_pool(name="h2p" + R, bufs=1))
        ppool = ctx.enter_context(tc.tile_pool(name="ppool" + R, bufs=1))
        epool = ctx.enter_context(tc.tile_pool(name="epool" + R, bufs=1))

        # issued from the ACT queue: their region-reuse WAR (they reuse the xT
        # region) must not block SP's later gather/output DMAs
        p1r = ppool.tile([128, NJ, PB], F16, name="p1r")
        nc.scalar.dma_start(p1r[:], p1T_in[:].rearrange("(g p) c -> p g c", p=128))
        p2r = ppool.tile([128, NJ, PB], F16, name="p2r")
        nc.scalar.dma_start(p2r[:], p2T_in[:].rearrange("(g p) c -> p g c", p=128))

        if STOP_AFTER == "prep":
            srow0 = npool.tile([1, PB], F32, tag="sr0")
            nc.vector.tensor_copy(srow0[:], mask_sb[0:1, 0, 0:PB])
            nc.sync.dma_start(scores_out[:], srow0[:])
            ctx.close()
            continue

        # ---- stage D: the 8 heads ----
        eluo = []
        for h in range(H):
            t = elup.tile([FH, IB], F16, name=f"eluo{h}")
            eluo.append(t)
        for h in range(H):
            attention(f"h{h}", f1b[h],
                      lambda jb, h=h: haug_lhsT(jb, h),
                      lambda jb, h=h: F2nat[:, jb, h:h + 1],
                      eluo[h],
                      skip_chunks=NCH if h == 0 else 0,
                      ps_hp=hp0 if h == 0 else None)

        if STOP_AFTER == "att1":
            srow0 = npool.tile([1, PB], F32, tag="sr0")
            nc.vector.tensor_copy(srow0[:], eluo[0][0:1, 0:PB])
            nc.sync.dma_start(scores_out[:], srow0[:])
            ctx.close()
            continue

        # ---- stage E: layer-2 h2 (local rows) + f12, then gather ----
        # h2loc cols: [h2(0:64) | ones(64) | A*f2_2(65)]
        h2loc = h2p.tile([128, SUB, FH + 2], F32, name="h2loc")
        nc.gpsimd.memset(h2loc[:, :, FH], 1.0)
        for s in range(SUB):
            ps_h2 = ps_small.tile([128, FH + 1], F32, tag="pss", name=f"psh2{s}")
            for h in range(H):
                nc.tensor.matmul(ps_h2[:], eluo[h][:, s * 128:(s + 1) * 128],
                                 Wor[:, h, 0:FH + 1],
                                 start=(h == 0), stop=(h == H - 1))
            nc.scalar.copy(h2loc[:, s, 0:FH], ps_h2[:, 0:FH])
            # f2_2 column pre-scaled by A for the Schraudolph op
            nc.scalar.mul(h2loc[:, s, FH + 1:FH + 2], ps_h2[:, FH:FH + 1], EXP_A)
        ps_f12 = ps_small.tile([1, IB], F32, tag="pss", name="psf12")
        for h in range(H):
            nc.tensor.matmul(ps_f12[:], Wor[:, h, FH + 1:FH + 2], eluo[h][:],
                             start=(h == 0), stop=(h == H - 1))
        # A-prescale f12 on the copy out of PSUM (fp16 for the K=1 broadcast)
        f12sb = h2p.tile([1, IB], F16, name="f12sb")
        nc.scalar.mul(f12sb[:], ps_f12[:], EXP_A)
        ps_fb = ps_small.tile([128, IB], F32, tag="pss", name="psfb")
        nc.tensor.matmul(ps_fb[:], ones16[0:1, 0:128], f12sb[:],
                         start=True, stop=True)
        f12b = h2p.tile([128, IB], F16, name="f12b")
        nc.scalar.copy(f12b[:], ps_fb[:])

        h2loc_d = dram.tile([IB, FH + 2], F32)
        nc.sync.dma_start(h2loc_d[:].rearrange("(s p) c -> p s c", p=128), h2loc[:])
        h2g_d = dram.tile([N, FH + 2], F32, addr_space="Shared")
        if SIM_NOCOLL:
            nc.sync.dma_start(h2g_d[0:IB, :], h2loc_d[:])
        else:
            nc.gpsimd.collective_compute(
                "AllGather", mybir.AluOpType.bypass, replica_groups=groups,
                ins=[h2loc_d[:].opt()], outs=[h2g_d[:].opt()])
        # split readback+cast in halves so layer-2 attention starts while the
        # second half is still in flight
        h2r = h2p.tile([128, NJ, FH + 2], F32, name="h2r")
        h2g_r = h2g_d[:].rearrange("(g p) c -> p g c", p=128)
        nc.sync.dma_start(h2r[:, 0:NJ // 2, :], h2g_r[:, 0:NJ // 2, :])
        nc.sync.dma_start(h2r[:, NJ // 2:NJ, :], h2g_r[:, NJ // 2:NJ, :])
        # bf16 view of [h2|1] for the layer-2 attention matmuls
        h2b = h2p.tile([128, NJ, FH + 1], BF16, name="h2b")
        nc.scalar.copy(h2b[:, 0:NJ // 2, :], h2r[:, 0:NJ // 2, 0:FH + 1])
        nc.scalar.copy(h2b[:, NJ // 2:NJ, :], h2r[:, NJ // 2:NJ, 0:FH + 1])

        # ---- stage F: layer-2 attention (single head) ----
        xoT = h2p.tile([FH, IB], F32, name="xoT")
        attention("l2", f12b,
                  lambda jb: h2b[:, jb, :],
                  lambda jb: h2r[:, jb, FH + 1:FH + 2],
                  xoT)

        # ---- stage G: x_out natural layout + gather ----
        xol = h2p.tile([128, SUB, FH], F16, name="xol")
        for s in range(SUB):
            ps_tr = ps_small.tile([128, FH], F32, tag="pss", name=f"pstr{s}")
            nc.tensor.transpose(ps_tr[:], xoT[:, s * 128:(s + 1) * 128],
                                ident[0:FH, 0:FH])
            nc.scalar.copy(xol[:, s, :], ps_tr[:])
        xo_d = dram.tile([IB, FH], F16)
        nc.sync.dma_start(xo_d[:].rearrange("(s p) c -> p s c", p=128), xol[:])
        xog_d = dram.tile([N, FH], F16, addr_space="Shared")
        if SIM_NOCOLL:
            nc.sync.dma_start(xog_d[0:IB, :], xo_d[:])
        else:
            nc.gpsimd.collective_compute(
                "AllGather", mybir.AluOpType.bypass, replica_groups=groups,
                ins=[xo_d[:].opt()], outs=[xog_d[:].opt()])
        xor_ = h2p.tile([128, NJ, FH], F16, name="xor")
        xog_r = xog_d[:].rearrange("(g p) c -> p g c", p=128)
        nc.sync.dma_start(xor_[:, 0:NJ // 2, :], xog_r[:, 0:NJ // 2, :])
        nc.sync.dma_start(xor_[:, NJ // 2:NJ, :], xog_r[:, NJ // 2:NJ, :])
        DBG["h2g"] = h2g_d
        DBG["xog"] = xog_d
        DBG["eluo"] = eluo
        DBG["haug0"] = haug_g[0]
        DBG["f1ball"] = f1ball
        DBG["F2nat"] = F2nat
        DBG["mask"] = mask_sb

        # ---- stage H: pair embeddings + scores ----
        ps_e1 = ps_small.tile([FH, PB], F32, tag="pss", name="ps_e1")
        for jb in range(NJ):
            nc.tensor.matmul(ps_e1[:], xor_[:, jb, :], p1r[:, jb, :],
                             start=(jb == 0), stop=(jb == NJ - 1))
        e1sb = epool.tile([FH, PB], F32)
        nc.scalar.copy(e1sb[:], ps_e1[:])
        ps_e2 = ps_small.tile([FH, PB], F32, tag="pss", name="ps_e2")
        for jb in range(NJ):
            nc.tensor.matmul(ps_e2[:], xor_[:, jb, :], p2r[:, jb, :],
                             start=(jb == 0), stop=(jb == NJ - 1))
        e2sb = epool.tile([FH, PB], F32)
        nc.scalar.copy(e2sb[:], ps_e2[:])

        ps_g = ps_small.tile([FH, PB], F32, tag="pss", name="ps_g")
        nc.tensor.matmul(ps_g[:], wgt[:], e1sb[:], start=True, stop=True)
        prod = epool.tile([FH, PB], F32)
        nc.vector.tensor_mul(prod[:], ps_g[:], e2sb[:])
        ps_s = ps_small.tile([1, PB], F32, tag="pss", name="ps_s")
        nc.tensor.matmul(ps_s[:], ones32[0:FH, 0:1], prod[:], start=True, stop=True)
        srow = epool.tile([1, PB], F32)
        nc.scalar.copy(srow[:], ps_s[:])
        nc.sync.dma_start(scores_out[:], srow[:])
        ctx.close()

    return nc


_CACHE = {}


def _get_nc(reps=1):
    key = f"nc{reps}"
    if key not in _CACHE:
        nc = bacc.Bacc(None, target_bir_lowering=False, debug=False, num_devices=NC)
        build(nc, reps=reps)
        nc.compile()
        _CACHE[key] = nc
    return _CACHE[key]


def prep_inputs(x, adj, pair1_map, pair2_map, Wh, a1h, a2h, W_out, a1_out,
                a2_out, weight):
    import concourse.mybir as _mb
    bf16 = _mb.dt.np(_mb.dt.bfloat16)
    x64 = np.asarray(x, np.float64)
    adj = np.asarray(adj)
    xT = np.ascontiguousarray(np.asarray(x, np.float32).T)             # [FIN, N]
    # mask pre-scaled by the Schraudolph A; any hugely-negative value works
    maskT = np.where(adj > 0, np.float32(0.0),
                     np.float32(EXP_A * MASKVAL)).T.astype(bf16)       # [j, i]
    Wall = np.ascontiguousarray(
        np.transpose(np.asarray(Wh, np.float64), (1, 0, 2)).reshape(FIN, H * FH)
    ).astype(np.float32)
    # layer-1 f1/f2 host-exact in fp64
    w1 = np.einsum("hkf,hf->kh", np.asarray(Wh, np.float64), np.asarray(a1h, np.float64))
    w2 = np.einsum("hkf,hf->kh", np.asarray(Wh, np.float64), np.asarray(a2h, np.float64))
    F1 = x64 @ w1                                                      # [N, H]
    F2 = x64 @ w2
    FT1 = np.ascontiguousarray((F1 * EXP_A).T.astype(np.float16))      # A*f1, [H, N]
    f2n = np.ascontiguousarray((F2 * EXP_A).astype(np.float32))        # A*f2, [N, H]
    # layer-2 weights, head-reordered: Wor[f, h, :] = [W_out[h*64+f] | w2o | w1o]
    w1o = np.asarray(W_out, np.float64) @ np.asarray(a1_out, np.float64)
    w2o = np.asarray(W_out, np.float64) @ np.asarray(a2_out, np.float64)
    Wof = np.concatenate([np.asarray(W_out, np.float64), w2o[:, None],
                          w1o[:, None]], axis=1)                       # [512, 66]
    Wor = np.ascontiguousarray(
        Wof.reshape(H, FH, FH + 2).transpose(1, 0, 2).reshape(FH, H * (FH + 2))
    ).astype(np.float16)
    p1T = np.ascontiguousarray(np.asarray(pair1_map, np.float16).T)    # [N, NPAIR]
    p2T = np.ascontiguousarray(np.asarray(pair2_map, np.float16).T)
    wgt = np.ascontiguousarray(np.asarray(weight, np.float32))

    in_maps = []
    for c in range(NC):
        i0, i1 = c * IB, (c + 1) * IB
        p0, p1 = c * PB, (c + 1) * PB
        in_maps.append({
            "xT_in": xT,
            "Wall_in": Wall,
            "Wor_in": Wor,
            "wgt_in": wgt,
            "maskT_in": np.ascontiguousarray(maskT[:, i0:i1]),
            "f1b_in": np.ascontiguousarray(np.broadcast_to(
                FT1[:, i0:i1].reshape(1, H * IB), (128, H * IB))),
            "f2n_in": f2n,
            "p1T_in": np.ascontiguousarray(p1T[:, p0:p1]),
            "p2T_in": np.ascontiguousarray(p2T[:, p0:p1]),
        })
    return in_maps


def run(inputs, trace=False, **kw):
    nc = _get_nc()
    in_maps = prep_inputs(**inputs)
    res = run_bass_kernel_spmd(nc, in_maps, list(range(NC)), trace=trace, **kw)
    scores = np.concatenate(
        [res.results[c]["scores_out"].reshape(-1) for c in range(NC)])
    return scores.astype(np.float32), res


def kernel(**inputs):
    return run(inputs)[0]


def _make_fn(nc, in_maps):
    import jax
    from jax.sharding import Mesh, PartitionSpec, NamedSharding
    from jax.experimental.shard_map import shard_map
    from concourse import bass2jax
    import concourse.mybir as _mb

    bass2jax.install_neuronx_cc_hook()
    partition_name = nc.partition_id_tensor.name if nc.partition_id_tensor else None
    in_names, out_names, out_avals, zero_outs = [], [], [], []
    for alloc in nc.m.functions[0].allocations:
        if not isinstance(alloc, _mb.MemoryLocationSet):
            continue
        name = alloc.memorylocations[0].name
        if alloc.kind == "ExternalInput":
            if name != partition_name:
                in_names.append(name)
        elif alloc.kind == "ExternalOutput":
            shape = list(alloc.tensor_shape)
            npdt = _mb.dt.np(alloc.dtype)
            out_names.append(name)
            out_avals.append(jax.core.ShapedArray(shape, npdt))
            zero_outs.append(np.zeros(shape, npdt))
    n_params = len(in_names)
    n_outs = len(out_names)
    all_in_names = list(in_names) + list(out_names)
    if partition_name is not None:
        all_in_names.append(partition_name)

    def _body(*args):
        operands = list(args)
        if partition_name is not None:
            operands.append(bass2jax.partition_id_tensor())
        outs = bass2jax._bass_exec_p.bind(
            *operands, out_avals=tuple(out_avals), in_names=tuple(all_in_names),
            out_names=tuple(out_names), lowering_input_output_aliases=(),
            sim_require_finite=True, sim_require_nnan=True, nc=nc)
        return tuple(outs)

    devices = jax.devices()[:NC]
    mesh = Mesh(np.asarray(devices), ("core",))
    in_specs = (PartitionSpec("core"),) * (n_params + n_outs)
    out_specs = (PartitionSpec("core"),) * n_outs
    fn = jax.jit(shard_map(_body, mesh=mesh, in_specs=in_specs,
                           out_specs=out_specs, check_rep=False),
                 keep_unused=True)
    concat_in = [
        np.concatenate([np.asarray(in_maps[c][nm]) for c in range(NC)], axis=0)
        for nm in in_names]
    concat_zeros = [np.zeros((NC * z.shape[0], *z.shape[1:]), z.dtype)
                    for z in zero_outs]
    sh = NamedSharding(mesh, PartitionSpec("core"))
    dev_in = [jax.device_put(a, sh) for a in concat_in]
    dev_zero = [jax.device_put(a, sh) for a in concat_zeros]
    return fn, dev_in, dev_zero


def bench(inputs, iters=6, kreps=5):
    """Device time per kernel pass, via the in-NEFF replication slope.

    Builds the program once with 1 rep and once with `kreps` reps of the
    whole computation; (t_k - t_1)/(k - 1) cancels the (large, stable) axon
    dispatch floor and yields per-pass device time.
    """
    import time
    import jax
    in_maps = prep_inputs(**inputs)
    fns = {}
    for reps in (1, kreps):
        nc = _get_nc(reps=reps)
        fn, dev_in, dev_zero = _make_fn(nc, in_maps)
        jax.block_until_ready(fn(*dev_in, *dev_zero))  # warm/compile
        fns[reps] = (fn, dev_in, dev_zero)

    def once(reps):
        fn, dev_in, dev_zero = fns[reps]
        t0 = time.perf_counter()
        jax.block_until_ready(fn(*dev_in, *dev_zero))
        return time.perf_counter() - t0

    t1s, tks, diffs = [], [], []
    for _ in range(3 * iters):
        a = once(1)
        b = once(kreps)
        c = once(1)
        t1s += [a, c]
        tks.append(b)
        diffs.append(b - (a + c) / 2)
    diffs.sort()
    med = diffs[len(diffs) // 2]
    out = {
        "t1_ns": min(t1s) * 1e9,
        f"t{kreps}_ns": min(tks) * 1e9,
        "pooled_med_ns": med / (kreps - 1) * 1e9,
        "per_exec_ns": max(med / (kreps - 1) * 1e9, 0.0),
    }
    return out


if __name__ == "__main__":
    # quick self-drive with random inputs of the right shapes (no reference)
    rng = np.random.default_rng(0)
    ins = dict(
        x=rng.standard_normal((N, FIN), dtype=np.float32),
        adj=(rng.random((N, N)) < 0.5).astype(np.int32),
        pair1_map=rng.standard_normal((NPAIR, N), dtype=np.float32),
        pair2_map=rng.standard_normal((NPAIR, N), dtype=np.float32),
        Wh=rng.standard_normal((H, FIN, FH), dtype=np.float32) * 0.1,
        a1h=rng.standard_normal((H, FH), dtype=np.float32) * 0.3,
        a2h=rng.standard_normal((H, FH), dtype=np.float32) * 0.3,
        W_out=rng.standard_normal((FIN, FH), dtype=np.float32) * 0.1,
        a1_out=rng.standard_normal((FH,), dtype=np.float32) * 0.3,
        a2_out=rng.standard_normal((FH,), dtype=np.float32) * 0.3,
        weight=rng.standard_normal((FH, FH), dtype=np.float32) * 0.1,
    )
    out = kernel(**ins)
    print("scores:", out.shape, out[:8])
